# revision 77
# baseline (speedup 1.0000x reference)
"""Teacher-forced decoder LSTM on 8 TRN2 NeuronCores.

Problem: B=256, T=32, V=10000, E=H=512 (fp32 in/out).
  step s in 0..30: x = embed[caps[:, s]]
                   gates = x@W_ih.T + h@W_hh.T + b     (i,f,g,o)
                   c = sig(f)*c + sig(i)*tanh(g); h = sig(o)*tanh(c)
                   out[s+1] = h@W_lin.T + b_lin
  out[0] = 0.  Output [T, B, V].

Sharding: data-parallel over batch, B_local=32 per core.

Layout: the whole recurrence runs in TRANSPOSED space. Gates are
computed as gatesT[4H, 32] via out[128,32] = W_chunk.T @ hT_chunk
matmuls, so the PE moving dimension is the batch (32), not the gate dim
(512); the cell update produces hT directly in the layout that both the
next step's matmuls and the logits GEMM consume - no per-step
transposes. Gate blocks are host-permuted to [g,i,f,o]; the g rows
carry a host-baked 2x scale so one sigmoid ACT op covers g|i|f and
tanh(g) = 2*sig(2g)-1 is reconstructed by a Pool tensor_scalar. The
cell runs in two H-halves so hT chunks 0/1 publish early and the next
step's k={0,1} matmuls overlap the second half.

Precision: h-side matmuls + logits GEMM in bf16 (1 cycle/row at any
moving size); x-side matmuls in fp8-e4m3 DoubleRow (2 K-chunks per
instruction at 0.5 cycles/row - x-side quantization error enters the
gates once through small embedding values and does not compound);
psum accumulation and cell state stay fp32. Logits are stored bf16 and
widened to fp32 on the host. Measured rel err ~4.7e-3.

Schedule: per step, 64 h-side MMs (critical path) + 32 DoubleRow x-MMs
pre-accumulated into one of 6 rotating psum banks 4-5 steps ahead (bias
pre-written into the bank by a DVE copy; a start=True matmul carries it
for the first 6 steps to define psum has_written state on silicon).
Logits chunks (2x250 vocab cols) are interleaved into recurrence gaps
as h m-tiles complete, 1/step from step 4 and 2/step from step 11;
the remaining vocab x m-tile chunks stream after the recurrence with
double-buffered bf16 W_lin super-chunks. Embeddings for m-tiles 2..7
are gathered on device (indirect DMA) and transposed by the DMA xbar;
m-tiles 0/1 are host-staged to cut the startup critical path.
"""
import numpy as np

B_FULL, T, V, E, H = 256, 32, 10000, 512, 512
NCORES = 8
BL = B_FULL // NCORES          # 32 batch rows per core
S = T - 1                      # 31 recurrent steps
M_TOK = S * BL                 # 992 token rows per core (t-major)
NMT = (M_TOK + 127) // 128     # 8 m-tiles (last has 96 rows)
G4 = 4 * H                     # 2048 gate dims
CW = 2000                      # vocab super-chunk width
NSUP = V // CW                 # 5 super-chunks
EC = 500                       # emit chunk width (CW // 4)

_CACHE = {}


def _build():
    import concourse.bacc as bacc
    import concourse.mybir as mybir
    from concourse.tile import TileContext
    import concourse.bass as bass

    f32 = mybir.dt.float32
    bf16 = mybir.dt.bfloat16
    f8 = mybir.dt.float8e4
    DR = mybir.MatmulPerfMode.DoubleRow
    i32 = mybir.dt.int32
    SIG = mybir.ActivationFunctionType.Sigmoid
    TANH = mybir.ActivationFunctionType.Tanh
    ADD = mybir.AluOpType.add
    MUL = mybir.AluOpType.mult

    nc = bacc.Bacc()

    emb_d = nc.dram_tensor("emb", [V, E], bf16, kind="ExternalInput")
    # wihT/whhT pre-arranged on host to [128, 4k x 2048]: k-chunk k at free
    # [2048k:2048(k+1)], gate blocks inside permuted to [g,i,f,o] order.
    wihT_d = nc.dram_tensor("wihT", [128, 4 * G4], f8, kind="ExternalInput")
    whhT_d = nc.dram_tensor("whhT", [128, 4 * G4], bf16, kind="ExternalInput")
    biasblk_d = nc.dram_tensor("biasblk", [128, 512], f32, kind="ExternalInput")
    bias16_d = nc.dram_tensor("bias16", [16, 128], bf16, kind="ExternalInput")
    sel16_d = nc.dram_tensor("sel16", [16, 512], bf16, kind="ExternalInput")
    h0T_d = nc.dram_tensor("h0T", [128, 128], bf16, kind="ExternalInput")
    tok_d = nc.dram_tensor("tok", [128, NMT], i32, kind="ExternalInput")
    # host-transposed embeddings for m-tiles 0/1 (startup critical path);
    # m-tiles 2..7 are gathered+transposed on device during the recurrence
    xt801_d = nc.dram_tensor("xt801", [128, 1024], f8, kind="ExternalInput")
    # wlinT pre-arranged to [128, 4k x 10000]: k-chunk k at [10000k:...]
    wlinT_d = nc.dram_tensor("wlinT", [128, 4 * V], bf16, kind="ExternalInput")
    blin_d = nc.dram_tensor("blin", [128, V], bf16, kind="ExternalInput")
    out_d = nc.dram_tensor("out", [M_TOK, V], bf16, kind="ExternalOutput")

    with TileContext(nc) as tc:
        with tc.tile_pool(name="const", bufs=1) as cp, \
             tc.tile_pool(name="state", bufs=1) as st, \
             tc.tile_pool(name="xst", bufs=2) as xst, \
             tc.tile_pool(name="wlp", bufs=2) as wlp, \
             tc.tile_pool(name="stg", bufs=4) as stp, \
             tc.tile_pool(name="rps", bufs=6, space="PSUM") as rps, \
             tc.tile_pool(name="p3ps", bufs=2, space="PSUM") as p3ps:

            # ---------- constant loads, spread across queues ----------
            # Startup critical path: x(0) needs xt01 + bias16/sel16 + wihT;
            # h(0) additionally needs whhT + h0T. Ws are split in quarters
            # across all 4 DMA-capable queues so each is resident ~2us after
            # its loads start.
            QW = G4  # quarter width of the [128, 4*G4] layout
            wihT = cp.tile([128, 4 * G4], f8, tag="wihT")
            whhT = cp.tile([128, 4 * G4], bf16, tag="whhT")
            tok_sb = cp.tile([128, NMT], i32, tag="tok_sb")
            biasblk = cp.tile([128, 512], f32, tag="biasblk")
            bias16 = cp.tile([16, 128], bf16, tag="bias16")
            sel16 = cp.tile([16, 512], bf16, tag="sel16")
            h0T = cp.tile([128, 128], bf16, tag="h0T")

            # xt[m]: transposed gathered embeddings for m-tile m,
            # E-chunk k at [128k:128(k+1)], token j at col j (4 steps x 32).
            # bf16 staging from the transpose DMA; fp8 copies feed the
            # DoubleRow x-side matmuls.
            xt = [st.tile([128, 512], bf16, tag=f"xt{m}", name=f"xt{m}")
                  for m in range(NMT)]
            xt8 = [st.tile([128, 512], f8, tag=f"xt8{m}", name=f"xt8{m}")
                   for m in range(NMT)]

            def wq(w_sb, w_d, q, eng):
                eng.dma_start(out=w_sb[:, QW * q:QW * (q + 1)],
                              in_=w_d[:, QW * q:QW * (q + 1)])

            # SP queue: host-staged fp8 xt for m-tiles 0/1, then W quarters
            nc.sync.dma_start(out=xt8[0][:], in_=xt801_d[:, 0:512])
            nc.sync.dma_start(out=xt8[1][:], in_=xt801_d[:, 512:1024])
            wq(wihT, wihT_d, 0, nc.sync)
            wq(whhT, whhT_d, 0, nc.sync)
            wq(wihT, wihT_d, 3, nc.sync)
            # ACT queue
            wq(wihT, wihT_d, 1, nc.scalar)
            wq(whhT, whhT_d, 1, nc.scalar)
            wq(whhT, whhT_d, 3, nc.scalar)
            # Pool queue
            nc.gpsimd.dma_start(out=biasblk[:], in_=biasblk_d[:])
            nc.gpsimd.dma_start(out=bias16[:], in_=bias16_d[:])
            nc.gpsimd.dma_start(out=sel16[:], in_=sel16_d[:])
            nc.gpsimd.dma_start(out=tok_sb[:], in_=tok_d[:])
            nc.gpsimd.dma_start(out=h0T[:], in_=h0T_d[:])
            wq(wihT, wihT_d, 2, nc.gpsimd)
            wq(whhT, whhT_d, 2, nc.gpsimd)
            # warm the sigmoid ACT table while ACT is idle (otherwise the
            # first sig of step 0 pays the 1.3us LoadActFuncSet mid-chain)
            wsc = cp.tile([1, 1], f32, tag="wsc")
            nc.scalar.activation(out=wsc[0:1, 0:1], in_=biasblk[0:1, 0:1],
                                 func=SIG)
            # wl0 split between Pool and SP queues - on ACT it would block
            # step 0's chain activations behind 6us of DMA
            wl0 = wlp.tile([128, 4 * CW], bf16, tag="wl", name="wl0")
            for k in range(2):
                nc.gpsimd.dma_start(out=wl0[:, CW * k:CW * (k + 1)],
                                    in_=wlinT_d[:, V * k:V * k + CW])
            for k in range(2, 4):
                nc.sync.dma_start(out=wl0[:, CW * k:CW * (k + 1)],
                                  in_=wlinT_d[:, V * k:V * k + CW])

            # ---------- state ----------
            # h_allT: transposed hidden states, chunk k at [992k:992(k+1)],
            # step s at cols 32s within each chunk. bf16; rhs of recurrence
            # MMs and lhsT of phase-3 MMs.
            h_allT = st.tile([128, 4 * M_TOK], bf16, tag="h_allT")
            cT = st.tile([128, 128], f32, tag="cT")
            nc.vector.memset(cT[:], 0.0)
            act_sb = st.tile([128, 512], f32, tag="act_sb")  # g|i|f|o blocks
            t1 = st.tile([128, 128], f32, tag="t1")
            tg = st.tile([128, 128], f32, tag="tg")
            t2 = st.tile([128, 128], f32, tag="t2")
            th = st.tile([128, 128], f32, tag="th")

            def gather(m):
                rows = min(128, M_TOK - 128 * m)
                gx = xst.tile([128, 512], bf16, tag="gx", name=f"gx{m}")
                nc.gpsimd.indirect_dma_start(
                    out=gx[0:rows, :], out_offset=None, in_=emb_d[:],
                    in_offset=bass.IndirectOffsetOnAxis(
                        ap=tok_sb[0:rows, m:m + 1], axis=0))
                # single chunked-transpose DMA: out[p, k, j] = gx[j, 128k+p]
                nc.sync.dma_start_transpose(
                    out=xt[m][:].rearrange("p (k j) -> p k j", k=4)[:, :, 0:rows],
                    in_=gx[0:rows, :])
                nc.gpsimd.tensor_copy(
                    out=xt8[m][:].rearrange("p (k j) -> p k j", k=4)[:, :, 0:rows],
                    in_=xt[m][:].rearrange("p (k j) -> p k j", k=4)[:, :, 0:rows])

            blin_sb = cp.tile([128, V], bf16, tag="blin_sb")
            nc.sync.dma_start(out=blin_sb[:], in_=blin_d[:])

            # ---------- recurrence helpers ----------
            pgs = {}

            def emit_x(s):
                """Bias init + x-side gate MMs for step s into a fresh psum
                bank. The bias is written by a Pool copy (not a PE matmul);
                all MMs then accumulate with start=False. Safe because every
                bank cycle writes all 512 columns, so no pending-zero bits
                survive from the previous user of the bank."""
                m, a = divmod(s, 4)
                pg = rps.tile([128, 512], f32, tag="pg", name=f"pg{s}")
                pgs[s] = pg
                if s < 6:
                    # first use of each psum bank: establish has_written via
                    # a start=True matmul carrying the bias (hw state of a
                    # fresh bank is undefined on real silicon)
                    nc.tensor.matmul(out=pg[:], lhsT=bias16[:], rhs=sel16[:],
                                     start=True, stop=False,
                                     skip_group_check=True)
                else:
                    nc.vector.tensor_copy(out=pg[:], in_=biasblk[:])
                wv = wihT[:].rearrange("p (kp i n) -> p kp i n", kp=2, i=2)
                xv = xt8[m][:].rearrange("p (kp i j) -> p kp i j", kp=2, i=2)
                for kp in range(2):
                    rhs = xv[:, kp, :, 32 * a:32 * a + 32]
                    for r in range(16):
                        nc.tensor.matmul(
                            out=pg[:, 32 * r:32 * r + 32],
                            lhsT=wv[:, kp, :, 128 * r:128 * (r + 1)],
                            rhs=rhs, start=False, stop=False,
                            perf_mode=DR, skip_group_check=True)
                return pg

            def emit_h(s, pg):
                # k-pair-major: the k={0,1} group only needs hT chunks 0/1,
                # which the previous step's first cell half produces early -
                # these MMs overlap with the previous step's second half.
                for kg in range(2):
                    for r in range(16):  # g blocks first within each group
                        for k in (2 * kg, 2 * kg + 1):
                            if s == 0:
                                rhs = h0T[:, 32 * k:32 * (k + 1)]
                            else:
                                c0 = M_TOK * k + 32 * (s - 1)
                                rhs = h_allT[:, c0:c0 + 32]
                            nc.tensor.matmul(
                                out=pg[:, 32 * r:32 * r + 32],
                                lhsT=whhT[:, G4 * k + 128 * r:G4 * k + 128 * (r + 1)],
                                rhs=rhs, start=False, stop=(k == 3),
                                skip_group_check=True)

            def emit_chunk(m, g0, w, wl_t, eng):
                """Logits for m-tile m, vocab cols [g0:g0+w] (within wl_t)."""
                rows = min(128, M_TOK - 128 * m)
                coff = g0 % CW
                pl = p3ps.tile([128, 512], f32, tag="pl")
                for k in range(4):
                    nc.tensor.matmul(
                        out=pl[0:rows, 0:w],
                        lhsT=h_allT[:, M_TOK * k + 128 * m:M_TOK * k + 128 * m + rows],
                        rhs=wl_t[:, CW * k + coff:CW * k + coff + w],
                        start=(k == 0), stop=(k == 3))
                stg = stp.tile([128, 512], bf16, tag="stg")
                eng.tensor_tensor(out=stg[0:rows, 0:w], in0=pl[0:rows, 0:w],
                                  in1=blin_sb[0:rows, g0:g0 + w], op=ADD)
                nc.sync.dma_start(out=out_d[128 * m:128 * m + rows, g0:g0 + w],
                                  in_=stg[0:rows, 0:w])

            # reshaped views for the strided hT write (4 chunks of 32 cols)
            hv = h_allT[:].rearrange("p (k c) -> p k c", k=4)
            ov = act_sb[:, 384:512].rearrange("p (k c) -> p k c", k=4)
            tv = th[:].rearrange("p (k c) -> p k c", k=4)

            # prologue: pre-accumulate bias+x for step 0 only (a deeper
            # prologue would sit ahead of step 0's h-MMs in the in-order PE
            # queue and delay the whole chain); the loop tops up to depth 5.
            emit_x(0)
            next_x = 1

            # emit pair schedule: (sup, m, c), consumed one per step from
            # step 4, two per step from step 16 (ready-frontier permitting).
            # sup0 pairs take priority; sup1 only after wl1 is loaded (s>=13).
            q0 = [(0, m, c) for m in range(7) for c in range(4)]
            q1 = [(1, m, c) for m in range(7) for c in range(4)]
            wl1 = None

            # ---------- recurrence ----------
            for s in range(S):
                pg = pgs.pop(s)
                emit_h(s, pg)
                # activations: one sigmoid over g|i|f (g rows carry a
                # host-baked 2x scale; tanh(g)=2*sig(2g)-1 is reconstructed
                # on Pool), then sigmoid over o
                nc.scalar.activation(out=act_sb[:, 0:384],
                                     in_=pg[:, 0:384], func=SIG)
                nc.scalar.activation(out=act_sb[:, 384:512],
                                     in_=pg[:, 384:512], func=SIG)
                # cell in two H-halves so hT chunks 0/1 are published
                # early for the next step's k={0,1} matmuls
                for hh in range(2):
                    lo, hi = 64 * hh, 64 * (hh + 1)
                    nc.gpsimd.tensor_scalar(out=tg[:, lo:hi],
                                            in0=act_sb[:, lo:hi],
                                            scalar1=2.0, scalar2=-1.0,
                                            op0=MUL, op1=ADD)
                    nc.gpsimd.tensor_tensor(out=t1[:, lo:hi],
                                            in0=act_sb[:, 128 + lo:128 + hi],
                                            in1=tg[:, lo:hi], op=MUL)
                    nc.gpsimd.tensor_tensor(out=t2[:, lo:hi],
                                            in0=act_sb[:, 256 + lo:256 + hi],
                                            in1=cT[:, lo:hi], op=MUL)
                    nc.gpsimd.tensor_tensor(out=cT[:, lo:hi], in0=t1[:, lo:hi],
                                            in1=t2[:, lo:hi], op=ADD)
                    nc.scalar.activation(out=th[:, lo:hi], in_=cT[:, lo:hi],
                                         func=TANH)
                    nc.gpsimd.tensor_tensor(
                        out=hv[:, 2 * hh:2 * hh + 2, 32 * s:32 * s + 32],
                        in0=act_sb[:, 384 + lo:384 + hi].rearrange(
                            "p (k c) -> p k c", k=2),
                        in1=th[:, lo:hi].rearrange("p (k c) -> p k c", k=2),
                        op=MUL)
                # -- interleaved logits chunks (sup0 + sup1), emitted as
                # two 250-wide halves at different PE-queue positions so the
                # filler granularity matches the chain gaps --
                halves = []
                if s >= 4:
                    frontier = (s - 4) // 4
                    nem = 1 if s < 11 else 2
                    for _ in range(nem):
                        if q0 and q0[0][1] <= frontier:
                            sup, em, ec = q0.pop(0)
                            halves.append((em, CW * sup + EC * ec, wl0))
                        elif s >= 11 and q1 and q1[0][1] <= frontier:
                            sup, em, ec = q1.pop(0)
                            halves.append((em, CW * sup + EC * ec, wl1))
                for em, g0, wlt in halves:
                    emit_chunk(em, g0, EC // 2, wlt, nc.vector)
                # -- background gathers for m-tiles 2..7 --
                if s % 2 == 0 and s // 2 + 2 < NMT:
                    gather(s // 2 + 2)
                for em, g0, wlt in halves:
                    emit_chunk(em, g0 + EC // 2, EC // 2, wlt, nc.vector)
                # -- prefetch the sup1 W_lin chunk once gathers are done --
                if s == 9:
                    wl1 = wlp.tile([128, 4 * CW], bf16, tag="wl", name="wl1")
                    for k in range(4):
                        nc.sync.dma_start(out=wl1[:, CW * k:CW * (k + 1)],
                                          in_=wlinT_d[:, V * k + CW:V * k + 2 * CW])
                # -- pre-accumulate x-side, up to 3/step, depth <= 5 --
                for _ in range(3):
                    if next_x < S and next_x <= s + 5:
                        emit_x(next_x)
                        next_x += 1

            # ---------- phase 3 tail ----------
            tail = (q0 + [(0, 7, c) for c in range(4)]
                    + q1 + [(1, 7, c) for c in range(4)])
            for sup in range(2, NSUP):
                tail += [(sup, m, c) for m in range(NMT) for c in range(4)]
            wl_map = {0: wl0, 1: wl1}

            def load_wl(sup):
                t = wlp.tile([128, 4 * CW], bf16, tag="wl", name=f"wl{sup}")
                for k in range(4):
                    nc.scalar.dma_start(
                        out=t[:, CW * k:CW * (k + 1)],
                        in_=wlinT_d[:, V * k + CW * sup:V * k + CW * (sup + 1)])
                wl_map[sup] = t

            load_wl(2)
            cur_sup = 0
            for i, (sup, m, c) in enumerate(tail):
                if sup != cur_sup:
                    cur_sup = sup
                    if sup + 1 < NSUP:
                        load_wl(sup + 1)
                emit_chunk(m, CW * sup + EC * c, EC, wl_map[sup], nc.vector)

    nc.compile()
    return nc


def _prep_host(caps, latent, embed, W_ih, W_hh, b_ih, b_hh, W_lin, b_lin):
    import ml_dtypes
    bf = ml_dtypes.bfloat16

    caps = np.asarray(caps).astype(np.int32)
    latent = np.asarray(latent, dtype=np.float32)
    # permute gate dim to [g, i, f, o] block order
    perm = np.r_[1024:1536, 0:512, 512:1024, 1536:2048]
    W_ih_p = np.asarray(W_ih, dtype=np.float32)[perm]       # [2048, 512]
    W_hh_p = np.asarray(W_hh, dtype=np.float32)[perm]
    bias_p = (np.asarray(b_ih, dtype=np.float32)
              + np.asarray(b_hh, dtype=np.float32))[perm]
    # g rows pre-scaled by 2: the kernel computes sigmoid over g|i|f in one
    # ACT op and reconstructs tanh(g) = 2*sigmoid(2g) - 1 on the Pool engine
    W_ih_p[0:512] *= 2.0
    W_hh_p[0:512] *= 2.0
    bias_p[0:512] *= 2.0

    def karrange(WT):  # [512, 2048] -> [128, 4*2048], k-chunk k at 2048k
        return np.ascontiguousarray(
            WT.reshape(4, 128, G4).transpose(1, 0, 2).reshape(128, 4 * G4))

    f8 = ml_dtypes.float8_e4m3fn
    emb = np.ascontiguousarray(np.asarray(embed, dtype=np.float32)).astype(bf)
    wihT = karrange(W_ih_p.T).astype(f8)
    whhT = karrange(W_hh_p.T).astype(bf)
    biasblk = np.ascontiguousarray(
        np.repeat(bias_p.reshape(16, 128).transpose(1, 0)[:, :, None],
                  32, axis=2).reshape(128, 512)).astype(np.float32)
    bias16 = np.ascontiguousarray(bias_p.reshape(16, 128)).astype(bf)
    sel16 = np.zeros((16, 512), dtype=np.float32)
    for r in range(16):
        sel16[r, 32 * r:32 * (r + 1)] = 1.0
    sel16 = sel16.astype(bf)
    wlinT = np.ascontiguousarray(
        np.asarray(W_lin, dtype=np.float32).T.reshape(4, 128, V)
        .transpose(1, 0, 2).reshape(128, 4 * V)).astype(bf)
    blin = np.ascontiguousarray(np.broadcast_to(
        np.asarray(b_lin, dtype=np.float32)[None, :], (128, V))).astype(bf)

    in_maps = []
    for c in range(NCORES):
        caps_sh = caps[c * BL:(c + 1) * BL]                 # [32, 32]
        tok_flat = caps_sh[:, :S].T.reshape(M_TOK)          # t-major [992]
        tok_pad = np.zeros(NMT * 128, dtype=np.int32)
        tok_pad[:M_TOK] = tok_flat
        tok = np.ascontiguousarray(tok_pad.reshape(NMT, 128).T)
        lat_sh = latent[c * BL:(c + 1) * BL]                # [32, 512]
        h0T = np.ascontiguousarray(
            lat_sh.T.reshape(4, 128, 32).transpose(1, 0, 2)
            .reshape(128, 128)).astype(bf)
        # host-transposed embeddings for m-tiles 0/1 (first 256 token rows):
        # xt layout [128, (k, j)]: E-chunk k at 128k, token col j
        x01 = np.asarray(emb)[tok_flat[:256]]               # [256, 512] bf16
        xt01 = np.ascontiguousarray(
            x01.T.reshape(4, 128, 2, 128).transpose(1, 2, 0, 3)
            .reshape(128, 1024)).astype(bf)
        xt801 = xt01.astype(f8)
        in_maps.append(dict(
            emb=emb, wihT=wihT, whhT=whhT, biasblk=biasblk,
            bias16=bias16, sel16=sel16,
            h0T=h0T, tok=tok, wlinT=wlinT, blin=blin, xt801=xt801,
        ))
    return in_maps


def kernel(caps, latent, embed, W_ih, W_hh, b_ih, b_hh, W_lin, b_lin):
    from concourse.bass_utils import run_bass_kernel_spmd

    if "nc" not in _CACHE:
        _CACHE["nc"] = _build()
    nc = _CACHE["nc"]

    in_maps = _prep_host(caps, latent, embed, W_ih, W_hh, b_ih, b_hh,
                         W_lin, b_lin)
    res = run_bass_kernel_spmd(nc, in_maps, core_ids=list(range(NCORES)))
    out = np.zeros((T, B_FULL, V), dtype=np.float32)
    for c in range(NCORES):
        shard = np.asarray(res.results[c]["out"]).astype(np.float32)
        out[1:, c * BL:(c + 1) * BL, :] = shard.reshape(S, BL, V)
    return out
